# revision 30
# baseline (speedup 1.0000x reference)
"""ContextQueryAttention (BiDAF-style) Trainium2 kernel, 8-core data-parallel.

Math (per batch):
  s[i,j]  = wq.q_j + wc.c_i + sum_d c_id * wcq_d * q_jd          (L1 x L2)
  s1      = softmax_i(s * mq_j + (1-mq_j)*NEG)                   (softmax over i)
  s2      = softmax_i(s * mp_i + (1-mp_i)*NEG)
  a       = s1 @ Q ;  b = s1 @ (s2^T @ C)      (reassociated, no L1xL1)
  out     = [C, a, C*a, C*b]                                      (L1 x 4D)

v4 design notes:
 - mask packing via host-side permutations ("sparse attention"):
   i-side: rows with mp_i=0 have E2=0 -> context rows permuted (unmasked
   first), e2/t process ceil(n1/128) of 16 i-tiles. j-side: cols with
   mq_j=0 have E1=1, z1=2048 exactly -> query axis permuted, e1/ab
   process ceil(n2/128) of 4 j-tiles; fully-masked tiles enter a/b as a
   rank-1 term v = sum_j (1-mq_j)*rhs_ab[j,:] (one m=1 matmul +
   partition_broadcast), folded into psab during evacuation. rz1 on
   masked tiles is exactly 1/2048 (memset).
 - all operand prep on host; inputs arrive as 4 consolidated DMAs per
   batch (small / cwc_row / esc-pack / tail-pack + c1bf), issued from
   four different engine queues so issue serialization doesn't delay
   the first matmul.
 - qwq cancels in both softmaxes; E1 needs no SHIFT (scale=mq), E2
   masked/pad rows underflow exp to exact 0 via bias2=-1000.
 - HAM warmup + keep-warm dummy matmuls bridge PE-idle windows.
 - out[:,0:128] = context via DRAM->DRAM DMA issued at t=0.
 - escore emits all e2 (MM,ACT) pairs before the e1 groups so e2n
   completes early and the t phase can interleave with the other
   batch's escore.
"""

import numpy as np
import ml_dtypes

import concourse.bass as bass
import concourse.mybir as mybir
import concourse.tile as tile
from concourse import bacc
from concourse import bass_utils

F32 = mybir.dt.float32
BF16 = mybir.dt.bfloat16
EXP = mybir.ActivationFunctionType.Exp
ADD = mybir.AluOpType.add
MULT = mybir.AluOpType.mult

B, L1, L2, D = 16, 2048, 512, 128
NCORES = 8
BPC = B // NCORES
NT1 = L1 // 128
NT2 = L2 // 128
SHIFT = 1000.0

N_WARM = 10
N_FILL = 12

BF = ml_dtypes.bfloat16


def _build_program(nt1p, nt2p):
    n1, n2 = nt1p * 128, nt2p * 128
    nmt = NT2 - nt2p                       # fully-masked j tiles
    W_ESC = n1 + L2 + L1                   # [cwtp | qtp | cwt]
    QTP_O, CWT_O = n1, n1 + L2
    W_TAIL = nt1p * 129 + L2 + (nmt + 1)   # [c1bp | qnbp | m0]
    QNB_O, M0_O = nt1p * 129, nt1p * 129 + L2

    nc = bacc.Bacc("TRN2", target_bir_lowering=False, debug=False)

    ctx_d = nc.dram_tensor("ctx", [BPC, L1, D], F32, kind="ExternalInput").ap()
    esc_d = nc.dram_tensor("escpack", [BPC, 128, W_ESC], BF16, kind="ExternalInput").ap()
    tail_d = nc.dram_tensor("tailpack", [BPC, 128, W_TAIL], BF16, kind="ExternalInput").ap()
    c1bf_d = nc.dram_tensor("c1bf", [BPC, 128, NT1, 128], BF16, kind="ExternalInput").ap()
    small_d = nc.dram_tensor("smallpack", [BPC, 128, nt1p + nt2p], F32, kind="ExternalInput").ap()
    cwcr_d = nc.dram_tensor("cwc_row", [BPC, 1, L1], F32, kind="ExternalInput").ap()
    out_d = nc.dram_tensor("out", [BPC, L1, 4 * D], F32, kind="ExternalOutput").ap()

    with tile.TileContext(nc) as tc:
        with (
            tc.tile_pool(name="const", bufs=1) as const,
            tc.tile_pool(name="inp", bufs=2) as inp,
            tc.tile_pool(name="mid", bufs=2) as mid,
            tc.tile_pool(name="stp", bufs=2) as stp,
            tc.tile_pool(name="outp", bufs=8) as outp,
            tc.tile_pool(name="psesc", bufs=3, space="PSUM") as psesc,
            tc.tile_pool(name="psab", bufs=3, space="PSUM") as psab_p,
            tc.tile_pool(name="psmix", bufs=2, space="PSUM") as psmix,
        ):
            warm_w = const.tile([128, 64], BF16)
            nc.vector.memset(warm_w, 0.03)
            ones1 = const.tile([1, 128], BF16)
            nc.vector.memset(ones1, 1.0)

            def warm(nwarm):
                for _ in range(nwarm):
                    ps = psmix.tile([64, 64], F32, tag="mix")
                    nc.tensor.matmul(ps, warm_w, warm_w, start=True, stop=True)

            S = [dict() for _ in range(BPC)]

            def ph_dma_esc(b):
                # DMA is ~283GB/s aggregate and ~135GB/s per HW queue; split
                # each critical pack into halves on two queues, strict
                # priority order, everything later stays behind
                s = S[b]
                s["esc"] = inp.tile([128, W_ESC], BF16, tag="esc", name=f"esc{b}")
                s["small"] = inp.tile([128, nt1p + nt2p], F32, tag="small", name=f"sm{b}")
                s["cwc_row"] = inp.tile([1, L1], F32, tag="cwc_row", name=f"cr{b}")
                # never issue from the scalar queue: the EXP table-load
                # blocks it until ~10us
                if b == 0:
                    nc.sync.dma_start(out=s["small"], in_=small_d[b])
                    nc.sync.dma_start(out=s["cwc_row"], in_=cwcr_d[b])
                    h = CWT_O // 2
                    nc.sync.dma_start(out=s["esc"][:, 0:h], in_=esc_d[b, :, 0:h])
                    nc.gpsimd.dma_start(
                        out=s["esc"][:, h:CWT_O], in_=esc_d[b, :, h:CWT_O]
                    )
                    h2 = CWT_O + L1 // 2
                    nc.sync.dma_start(
                        out=s["esc"][:, CWT_O:h2], in_=esc_d[b, :, CWT_O:h2]
                    )
                    nc.gpsimd.dma_start(out=s["esc"][:, h2:], in_=esc_d[b, :, h2:])
                else:
                    nc.sync.dma_start(out=s["small"], in_=small_d[b])
                    nc.sync.dma_start(out=s["cwc_row"], in_=cwcr_d[b])
                    nc.sync.dma_start(
                        out=s["esc"][:, 0:CWT_O], in_=esc_d[b, :, 0:CWT_O]
                    )
                    nc.gpsimd.dma_start(
                        out=s["esc"][:, CWT_O:], in_=esc_d[b, :, CWT_O:]
                    )

            def ph_dma_tail(b):
                s = S[b]
                s["tail"] = inp.tile([128, W_TAIL], BF16, tag="tail", name=f"tl{b}")
                nc.sync.dma_start(out=s["tail"], in_=tail_d[b])
                s["c1bf"] = inp.tile([128, NT1, 128], BF16, tag="c1bf", name=f"cb{b}")
                nc.sync.dma_start(out=s["c1bf"], in_=c1bf_d[b])

            def ph_dma_ctx(b):
                # out[:, 0:128] = context, DRAM->DRAM on the sync queue so
                # its transfer is served after the input packs (issue order)
                # and before the output stores
                nc.sync.dma_start(out=out_d[b, :, 0:128], in_=ctx_d[b])

            def ph_prep(b):
                s = S[b]
                cwc_bc = mid.tile([128, L1], F32, tag="cwc_bc", name=f"cwb{b}")
                for nn in range(4):
                    nc.gpsimd.partition_broadcast(
                        cwc_bc[:, 512 * nn : 512 * (nn + 1)],
                        s["cwc_row"][:, 512 * nn : 512 * (nn + 1)],
                    )
                s["cwc_bc"] = cwc_bc
                rz1 = mid.tile([128, NT2], F32, tag="rz1", name=f"rz1{b}")
                if nt2p < NT2:
                    nc.vector.memset(rz1[:, nt2p:NT2], 1.0 / 2048.0)
                s["rz1"] = rz1
                s["e2n"] = mid.tile([128, nt1p, L2], BF16, tag="e2n", name=f"e2n{b}")
                s["e1"] = mid.tile([128, nt2p, L1], BF16, tag="e1", name=f"e1_{b}")
                s["z1"] = mid.tile([128, nt2p], F32, tag="z1", name=f"z1_{b}")

            def ph_esc_e2(b, its):
                s = S[b]
                for it in its:
                    pss = psesc.tile([128, 512], F32, tag="esc")
                    nc.tensor.matmul(
                        pss, s["esc"][:, it * 128 : (it + 1) * 128],
                        s["esc"][:, QTP_O : QTP_O + L2],
                        start=True, stop=True,
                    )
                    nc.scalar.activation(
                        s["e2n"][:, it, :], pss, EXP,
                        bias=s["small"][:, it : it + 1],
                    )

            def ph_esc_e1(b, jts):
                s = S[b]
                for jt in jts:
                    st_sb = stp.tile([128, L1], BF16, tag="st_sb")
                    for nn in range(4):
                        psst = psesc.tile([128, 512], F32, tag="esc")
                        nc.tensor.matmul(
                            psst,
                            s["esc"][:, QTP_O + jt * 128 : QTP_O + (jt + 1) * 128],
                            s["esc"][:, CWT_O + 512 * nn : CWT_O + 512 * (nn + 1)],
                            start=True, stop=True,
                        )
                        nc.vector.tensor_tensor(
                            st_sb[:, 512 * nn : 512 * (nn + 1)], psst,
                            s["cwc_bc"][:, 512 * nn : 512 * (nn + 1)], ADD,
                        )
                    nc.scalar.activation(
                        s["e1"][:, jt, :], st_sb, EXP,
                        scale=s["small"][:, nt1p + jt : nt1p + jt + 1],
                        accum_out=s["z1"][:, jt : jt + 1],
                    )

            def ph_t(b, jts):
                s = S[b]
                if 0 in jts:
                    s["rhs_ab"] = mid.tile(
                        [128, NT2, 256], BF16, tag="rhs_ab", name=f"rab{b}"
                    )
                for jt in jts:
                    if jt < nt2p:
                        nc.vector.reciprocal(
                            s["rz1"][:, jt : jt + 1], s["z1"][:, jt : jt + 1]
                        )
                    pst = psmix.tile([128, 129], F32, tag="mix")
                    for it in range(nt1p):
                        nc.tensor.matmul(
                            pst, s["e2n"][:, it, jt * 128 : (jt + 1) * 128],
                            s["tail"][:, it * 129 : (it + 1) * 129],
                            start=(it == 0), stop=(it == nt1p - 1),
                        )
                    rz2 = mid.tile([128, 1], F32, tag="rz2")
                    nc.vector.reciprocal(rz2, pst[:, 128:129])
                    rz12 = mid.tile([128, 1], F32, tag="rz12")
                    nc.vector.tensor_mul(rz12, rz2, s["rz1"][:, jt : jt + 1])
                    nc.vector.tensor_scalar_mul(
                        s["rhs_ab"][:, jt, 128:256], pst[:, 0:128], rz12
                    )
                    nc.vector.tensor_scalar_mul(
                        s["rhs_ab"][:, jt, 0:128],
                        s["tail"][:, QNB_O + jt * 128 : QNB_O + (jt + 1) * 128],
                        s["rz1"][:, jt : jt + 1],
                    )

            def ph_v(b):
                """v = sum_j (1-mq_j)*rhs_ab[j,:] over the fully-masked j
                tiles, staged as a duplicated [1, 512] bf16 row so one K=1
                N=512 matmul seeds both halves of a psab pair bank."""
                s = S[b]
                v_sb2 = mid.tile([1, 2, 256], BF16, tag="v_sb2", name=f"vsb{b}")
                if nmt > 0:
                    psv = psmix.tile([1, 256], F32, tag="mix")
                    for k in range(nmt):
                        jt = nt2p + k
                        nc.tensor.matmul(
                            psv, s["tail"][:, M0_O + k : M0_O + k + 1],
                            s["rhs_ab"][:, jt, :],
                            start=(k == 0), stop=(k == nmt - 1),
                        )
                    nc.vector.tensor_copy(v_sb2[:, 0, :], psv)
                    nc.vector.tensor_copy(v_sb2[:, 1, :], psv)
                else:
                    nc.vector.memset(v_sb2, 0.0)
                s["v_sb2"] = v_sb2

            def ph_ab(b, pairs, defer=False):
                s = S[b]
                for pair in pairs:
                    o_sb = outp.tile([128, 2, 384], F32, tag="o_sb", name=f"o{b}_{pair}")
                    ab_sb = outp.tile([128, 2, 256], BF16, tag="ab_sb", name=f"s{b}_{pair}")
                    s[f"o{pair}"], s[f"s{pair}"] = o_sb, ab_sb
                    # both halves of the pair accumulate in ONE psum bank:
                    # the K=1 v-matmul (start) seeds v everywhere, then the
                    # per-half e1 chains accumulate on top
                    ps = psab_p.tile([128, 2, 256], F32, tag="ab")
                    nc.tensor.matmul(
                        ps, ones1, s["v_sb2"], start=True, stop=False,
                        skip_group_check=True,
                    )
                    for half in range(2):
                        it = 2 * pair + half
                        for jt in range(nt2p):
                            nc.tensor.matmul(
                                ps[:, half, :],
                                s["e1"][:, jt, it * 128 : (it + 1) * 128],
                                s["rhs_ab"][:, jt, :],
                                start=False,
                                stop=(half == 1 and jt == nt2p - 1),
                                skip_group_check=True,
                            )
                    # during ab(b0) the DVE is the critical chain (b1's
                    # st_sb adds gate the last exps) — keep it light there
                    if b == 0:
                        nc.scalar.copy(ab_sb, ps)
                    else:
                        nc.vector.tensor_copy(ab_sb, ps)
                    mul1_eng = nc.gpsimd if b == 0 else nc.vector
                    mul1_eng.tensor_tensor(
                        o_sb[:, :, 128:256],
                        s["c1bf"][:, 2 * pair : 2 * pair + 2, :],
                        ab_sb[:, :, 0:128], MULT,
                    )
                    nc.gpsimd.tensor_tensor(
                        o_sb[:, :, 256:384],
                        s["c1bf"][:, 2 * pair : 2 * pair + 2, :],
                        ab_sb[:, :, 128:256], MULT,
                    )
                    if b == 1:
                        # keep the PE's activity monitor warm through the
                        # evacuation-paced tail (cold PE doubles chain time)
                        warm(2)
                    if not defer:
                        ph_ab_finish(b, [pair])

            def ph_ab_finish(b, pairs):
                # a-copy + store, deferrable so the scalar queue stays
                # exp-dense while escore work remains
                s = S[b]
                for pair in pairs:
                    o_sb, ab_sb = s[f"o{pair}"], s[f"s{pair}"]
                    nc.scalar.copy(o_sb[:, :, 0:128], ab_sb[:, :, 0:128])
                    nc.sync.dma_start(
                        out=out_d[b, pair * 256 : (pair + 1) * 256, 128:512].rearrange(
                            "(t p) f -> p t f", p=128
                        ),
                        in_=o_sb,
                    )

            warm(N_WARM)
            ph_dma_esc(0)
            ph_dma_esc(1)
            ph_dma_tail(0)
            ph_dma_tail(1)
            ph_dma_ctx(0)
            ph_dma_ctx(1)
            ph_prep(0)
            ph_esc_e2(0, range(nt1p))
            ph_prep(1)
            ph_esc_e2(1, range(0, 3))
            ph_esc_e1(0, range(nt2p))
            ph_t(0, [0, 1, 2, 3])
            ph_v(0)
            ph_esc_e2(1, range(3, nt1p))
            ph_esc_e1(1, [0])
            ph_t(1, [0])
            ph_ab(0, range(0, 3), defer=True)
            ph_esc_e1(1, [1])
            ph_t(1, [1])
            ph_ab(0, range(3, 6), defer=True)
            ph_esc_e1(1, range(2, nt2p))
            ph_t(1, [2, 3])
            ph_v(1)
            ph_ab(0, range(6, 8), defer=True)
            ph_ab_finish(0, range(0, 8))
            warm(N_FILL)
            ph_ab(1, range(0, 8))

    nc.compile()
    return nc


_NC_CACHE = {}
_NC_LAST = None


def _get_nc(nt1p=None, nt2p=None):
    global _NC_LAST
    if nt1p is None:
        if _NC_LAST is not None:
            return _NC_LAST
        nt1p, nt2p = 9, 3
    key = (nt1p, nt2p)
    if key not in _NC_CACHE:
        _NC_CACHE[key] = _build_program(nt1p, nt2p)
    _NC_LAST = _NC_CACHE[key]
    return _NC_LAST


def _make_in_maps(inputs):
    context = np.asarray(inputs["context"], dtype=np.float32)
    query = np.asarray(inputs["query"], dtype=np.float32)
    w = np.asarray(inputs["w"], dtype=np.float32)
    mp = np.asarray(inputs["mask_p"]).astype(np.float32)
    mq = np.asarray(inputs["mask_q"]).astype(np.float32)
    wc, wcq = w[D : 2 * D], w[2 * D :]

    n1max = int(mp.sum(axis=1).max())
    n2max = int(mq.sum(axis=1).max())
    nt1p = min(NT1, max(1, -(-n1max // 128)))
    nt2p = min(NT2, max(1, -(-n2max // 128)))
    n1, n2 = nt1p * 128, nt2p * 128
    nmt = NT2 - nt2p

    in_maps = []
    for c in range(NCORES):
        m = {k: [] for k in ("ctx", "escpack", "tailpack", "c1bf",
                             "smallpack", "cwc_row")}
        for bb in range(c * BPC, (c + 1) * BPC):
            ctx, qry = context[bb], query[bb]
            mpb, mqb = mp[bb], mq[bb]
            pi = np.argsort(-mpb, kind="stable")[:n1]
            pj = np.argsort(-mqb, kind="stable")
            cwc = ctx @ wc
            cp = ctx[pi]
            qp = qry[pj]
            # escpack = [cwtp | qtp | cwt], all [128, .] bf16
            esc = np.concatenate(
                [(cp * wcq).T, qp.T, (ctx * wcq).T], axis=1
            ).astype(BF)
            # tailpack = [c1bp | qnbp | m0]
            c1bp = np.concatenate(
                [cp.astype(BF).astype(np.float32), np.ones((n1, 1), np.float32)],
                axis=1,
            ).reshape(nt1p, 128, 129).transpose(1, 0, 2).reshape(128, nt1p * 129)
            qnbp = qp.reshape(NT2, 128, D).transpose(1, 0, 2).reshape(128, L2)
            m0 = np.zeros((128, nmt + 1), np.float32)
            if nmt > 0:
                m0[:, 0:nmt] = (1.0 - mqb[pj][n2:]).reshape(nmt, 128).T
            tailp = np.concatenate(
                [c1bp.astype(np.float32), qnbp, m0], axis=1
            ).astype(BF)
            bias2 = mpb[pi] * (cwc[pi] + SHIFT) - SHIFT
            small = np.concatenate(
                [bias2.reshape(nt1p, 128).T, mqb[pj][:n2].reshape(nt2p, 128).T],
                axis=1,
            ).astype(np.float32)
            m["ctx"].append(ctx)
            m["escpack"].append(esc)
            m["tailpack"].append(tailp)
            m["c1bf"].append(ctx.reshape(NT1, 128, D).transpose(1, 0, 2).astype(BF))
            m["smallpack"].append(small)
            m["cwc_row"].append(cwc[None, :])
        in_maps.append(
            {k: np.ascontiguousarray(np.stack(v)) for k, v in m.items()}
        )
    return in_maps, nt1p, nt2p


def kernel(context, query, w, mask_p, mask_q):
    in_maps, nt1p, nt2p = _make_in_maps(
        {"context": context, "query": query, "w": w, "mask_p": mask_p, "mask_q": mask_q}
    )
    nc = _get_nc(nt1p, nt2p)
    res = bass_utils.run_bass_kernel_spmd(nc, in_maps, core_ids=list(range(NCORES)))
    return np.concatenate([res.results[c]["out"] for c in range(NCORES)], axis=0)
